# revision 9
# baseline (speedup 1.0000x reference)
"""Trainium2 Bass kernel for nn_Attention_28930899706081 (sparse_attention).

Reference computation:
  k1 = l2norm_c(Wqk @ fmap1), k2 = l2norm_c(Wqk @ fmap2), q = l2norm_c(Wqk @ dmap)
  sim_i = q^T k_i per batch  -> [b, n, n] with n = h*w = 4096
  attn_i = softmax(sim_i, axis=-1)[:, None]  -> [b, 1, n, n]
  returns (attn1, attn2)

Sharding: 8 cores; core i handles batch b = i//4 and query-row block r = i%4
(1024 of 4096 rows). Each core computes the full normalized K for its batch
(recompute instead of collectives) and its row block of both sims + softmax.

ScalarE (ACT) is the bottleneck: the 8.4M softmax exps per core stream at
1 elem/cycle/lane @1.2GHz (~65us minimum, measured 2.03us/2048-chunk with the
read-accumulator hidden under the next exp's pipe fill). The design keeps that
conveyor saturated from as early as possible:
  - ONE ACT table set for the whole kernel (natural_log_exp_and_others):
    column rsqrt is exp(-0.5*ln(n2)) instead of Abs_reciprocal_sqrt, so norm
    work interleaves freely with softmax exps (no 2.7us table switches).
    The set is primed by a dummy ln at t~0.
  - q + k1 norms run up front (short ramp); k2's entire norm chain overlaps
    phase B1, borrowing sim-psum tiles for its proj/n2 matmuls.
  - squares/copies/normalize muls on VectorE; row sums from the ACT exp
    accumulator; attn normalize mul on VectorE; output fp16.
|sim| <= 1 because q/k are unit vectors, so softmax needs no max subtraction.
"""

import numpy as np
import ml_dtypes

B, C, H, W, D = 2, 256, 64, 64, 128
N = H * W  # 4096
QBLK = N // 4  # 1024 query rows per core
N_CORES = 8

_cached = {}


def _build():
    import concourse.mybir as mybir
    import concourse.tile as tile
    from concourse import bacc
    from contextlib import ExitStack

    f32 = mybir.dt.float32
    f16 = mybir.dt.float16
    bf16 = mybir.dt.bfloat16
    AF = mybir.ActivationFunctionType

    nc = bacc.Bacc(
        "TRN2",
        target_bir_lowering=False,
        debug=False,
        enable_asserts=False,
        num_devices=N_CORES,
    )

    f1_ext = nc.dram_tensor("f1", [C, N], bf16, kind="ExternalInput").ap()
    f2_ext = nc.dram_tensor("f2", [C, N], bf16, kind="ExternalInput").ap()
    xq_ext = nc.dram_tensor("xq", [C, QBLK], bf16, kind="ExternalInput").ap()
    wqkT_ext = nc.dram_tensor("wqkT", [C, D], bf16, kind="ExternalInput").ap()
    out_ext = nc.dram_tensor("out", [2, QBLK, N], f16, kind="ExternalOutput").ap()

    XCH = 1024  # phase A chunk (proj psum [128,1024] = 2 banks)
    CH = 2048  # phase B sim/exp chunk ([128,2048] = 4 banks)

    with tile.TileContext(nc) as tc, ExitStack() as ctx:
        consts = ctx.enter_context(tc.tile_pool(name="consts", bufs=1))
        xin = ctx.enter_context(tc.tile_pool(name="xin", bufs=8))
        ysq_pool = ctx.enter_context(tc.tile_pool(name="ysq", bufs=3))
        ybf_pool = ctx.enter_context(tc.tile_pool(name="ybf", bufs=3))
        ln_pool = ctx.enter_context(tc.tile_pool(name="lnp", bufs=2))
        rk_pool = ctx.enter_context(tc.tile_pool(name="rk", bufs=3))
        kn_pool = ctx.enter_context(tc.tile_pool(name="kn", bufs=1))
        e_pool = ctx.enter_context(tc.tile_pool(name="epool", bufs=6))
        attn_pool = ctx.enter_context(tc.tile_pool(name="attn", bufs=2))
        stat_pool = ctx.enter_context(tc.tile_pool(name="stat", bufs=4))

        # constants
        wqkT_sb = [
            consts.tile([128, D], bf16, tag=f"wqkT{k}", name=f"wqkT{k}")
            for k in range(2)
        ]
        nc.sync.dma_start(out=wqkT_sb[0][:], in_=wqkT_ext[0:128, :])
        nc.sync.dma_start(out=wqkT_sb[1][:], in_=wqkT_ext[128:256, :])
        ones_sb = consts.tile([128, 128], bf16, tag="ones", name="ones")
        nc.vector.memset(ones_sb[:], 1.0)
        # prime the single ACT table set (natural_log_exp_and_others) at t~0
        warm = consts.tile([128, 1], f32, tag="warm", name="warm")
        nc.scalar.activation(out=warm[:], in_=ones_sb[:, 0:1], func=AF.Ln)

        def rsqrt_cols(rk_out, nps, width):
            """rk = n2^-0.5 via exp(-0.5*ln(n2)) — stays in the exp table set."""
            tmp = ln_pool.tile([128, width], f32, tag=f"ln{width}", name="lntmp")
            nc.scalar.activation(out=tmp[:], in_=nps[:, 0:width], func=AF.Ln)
            nc.scalar.activation(out=rk_out, in_=tmp[:], func=AF.Exp, scale=-0.5)

        def norm_chunk(proj_psum, n2_psum, x_lo, x_hi, xn, h0, width, ptag=None):
            """project + column-l2-normalize one [128, width] chunk."""
            ps = proj_psum.tile([128, width], f32, tag=ptag or f"proj{width}", name="pps")
            for c in range(width // 512):
                sl = slice(c * 512, (c + 1) * 512)
                nc.tensor.matmul(
                    ps[:, sl], wqkT_sb[0][:], x_lo[:, sl], start=True, stop=False
                )
                nc.tensor.matmul(
                    ps[:, sl], wqkT_sb[1][:], x_hi[:, sl], start=False, stop=True
                )
            y_bf = ybf_pool.tile([128, width], bf16, tag=f"ybf{width}", name="y_bf")
            nc.vector.tensor_copy(y_bf[:], ps[:])
            ysq = ysq_pool.tile([128, width], bf16, tag=f"ysq{width}", name="ysq")
            nc.vector.tensor_mul(ysq[:], y_bf[:], y_bf[:])
            nps = n2_psum.tile([128, width], f32, tag=ptag or f"n2{width}", name="nps")
            for c in range(width // 512):
                sl = slice(c * 512, (c + 1) * 512)
                nc.tensor.matmul(
                    nps[:, sl], ones_sb[:], ysq[:, sl], start=True, stop=True
                )
            rk = rk_pool.tile([128, width], f16, tag=f"rk{width}", name="rk")
            rsqrt_cols(rk[:], nps, width)
            nc.vector.tensor_mul(xn[:, h0 : h0 + width], y_bf[:], rk[:])

        def load_x(x_ext, h0, width):
            x_lo = xin.tile([128, width], bf16, tag=f"xin{width}", name="x_lo")
            x_hi = xin.tile([128, width], bf16, tag=f"xin{width}", name="x_hi")
            nc.sync.dma_start(out=x_lo[:], in_=x_ext[0:128, h0 : h0 + width])
            nc.sync.dma_start(out=x_hi[:], in_=x_ext[128:256, h0 : h0 + width])
            return x_lo, x_hi

        qn = kn_pool.tile([128, QBLK], bf16, tag="qn", name="qn")
        k1n = kn_pool.tile([128, N], bf16, tag="k1n", name="k1n")
        k2n = kn_pool.tile([128, N], bf16, tag="k2n", name="k2n")

        with tc.tile_pool(name="proj_psum", bufs=2, space="PSUM") as proj_psum, \
             tc.tile_pool(name="n2_psum", bufs=2, space="PSUM") as n2_psum:
            x_lo, x_hi = load_x(xq_ext, 0, XCH)
            norm_chunk(proj_psum, n2_psum, x_lo, x_hi, qn, 0, XCH)
            for h in range(N // XCH):
                x_lo, x_hi = load_x(f1_ext, h * XCH, XCH)
                norm_chunk(proj_psum, n2_psum, x_lo, x_hi, k1n, h * XCH, XCH)

        with tc.tile_pool(name="sim_psum", bufs=2, space="PSUM") as sim_psum:

            def k2_group(g):
                """norm chain for k2 cols [g*CH, (g+1)*CH), on sim-psum tiles.

                Runs interleaved with phase B1's sims: same psum pool, and its
                ACT ops (ln/exp) slot into the exp conveyor without a table
                switch.
                """
                x_lo, x_hi = load_x(f2_ext, g * CH, CH)
                norm_chunk(sim_psum, sim_psum, x_lo, x_hi, k2n, g * CH, CH, ptag="sim")

            def phase_b(kn, s, interleave=()):
                """row block of sim + softmax for one K map, streamed to out[s]."""
                for t in range(QBLK // 128):
                    lhsT = qn[:, t * 128 : (t + 1) * 128]
                    attn = attn_pool.tile([128, N], f16, tag="attn", name="attn")
                    stile = stat_pool.tile([128, 2], f32, tag="stile", name="stile")
                    e_chunks = []
                    for j in range(N // CH):
                        ps = sim_psum.tile([128, CH], f32, tag="sim", name="sim_ps")
                        for c in range(CH // 512):
                            csl = slice(j * CH + c * 512, j * CH + (c + 1) * 512)
                            nc.tensor.matmul(
                                ps[:, c * 512 : (c + 1) * 512],
                                lhsT,
                                kn[:, csl],
                                start=True,
                                stop=True,
                            )
                        e = e_pool.tile([128, CH], f16, tag="e", name="e")
                        nc.scalar.activation(
                            out=e[:],
                            in_=ps[:],
                            func=AF.Exp,
                            accum_out=stile[:, j : j + 1],
                        )
                        e_chunks.append(e)
                    ssum = stat_pool.tile([128, 1], f32, tag="ssum", name="ssum")
                    nc.vector.reduce_sum(ssum[:], stile[:], axis=mybir.AxisListType.X)
                    recip = stat_pool.tile([128, 1], f32, tag="recip", name="recip")
                    nc.vector.reciprocal(recip[:], ssum[:])
                    for j, e in enumerate(e_chunks):
                        nc.vector.tensor_scalar_mul(
                            attn[:, j * CH : (j + 1) * CH], e[:], recip[:]
                        )
                        nc.sync.dma_start(
                            out=out_ext[
                                s, t * 128 : (t + 1) * 128, j * CH : (j + 1) * CH
                            ],
                            in_=attn[:, j * CH : (j + 1) * CH],
                        )
                    if t in interleave:
                        k2_group(list(interleave).index(t))

            phase_b(k1n, 0, interleave=(0, 1))
            phase_b(k2n, 1)

    nc.compile()
    return nc


def _get_nc():
    if "nc" not in _cached:
        _cached["nc"] = _build()
    return _cached["nc"]


def _in_maps(fmap1, fmap2, dmap, Wqk):
    bf = ml_dtypes.bfloat16
    f1r = np.asarray(fmap1, dtype=np.float32).reshape(B, C, N)
    f2r = np.asarray(fmap2, dtype=np.float32).reshape(B, C, N)
    dqr = np.asarray(dmap, dtype=np.float32).reshape(B, C, N)
    wT = np.ascontiguousarray(np.asarray(Wqk, dtype=np.float32).T).astype(bf)

    in_maps = []
    for i in range(N_CORES):
        b, r = divmod(i, 4)
        in_maps.append(
            {
                "f1": np.ascontiguousarray(f1r[b]).astype(bf),
                "f2": np.ascontiguousarray(f2r[b]).astype(bf),
                "xq": np.ascontiguousarray(
                    dqr[b][:, r * QBLK : (r + 1) * QBLK]
                ).astype(bf),
                "wqkT": wT,
            }
        )
    return in_maps


def kernel(fmap1, fmap2, dmap, Wqk):
    from concourse.bass_utils import run_bass_kernel_spmd

    in_maps = _in_maps(fmap1, fmap2, dmap, Wqk)
    nc = _get_nc()
    res = run_bass_kernel_spmd(nc, in_maps, core_ids=list(range(N_CORES)))
    _cached["last_result"] = res

    attn1 = np.empty((B, 1, N, N), dtype=np.float32)
    attn2 = np.empty((B, 1, N, N), dtype=np.float32)
    for i in range(N_CORES):
        b, r = divmod(i, 4)
        o = res.results[i]["out"]
        attn1[b, 0, r * QBLK : (r + 1) * QBLK, :] = o[0].astype(np.float32)
        attn2[b, 0, r * QBLK : (r + 1) * QBLK, :] = o[1].astype(np.float32)
    return (attn1, attn2)


# revision 10
# speedup vs baseline: 1.1429x; 1.1429x over previous
"""Trainium2 Bass kernel for nn_Attention_28930899706081 (sparse_attention).

Reference computation:
  k1 = l2norm_c(Wqk @ fmap1), k2 = l2norm_c(Wqk @ fmap2), q = l2norm_c(Wqk @ dmap)
  sim_i = q^T k_i per batch  -> [b, n, n] with n = h*w = 4096
  attn_i = softmax(sim_i, axis=-1)[:, None]  -> [b, 1, n, n]
  returns (attn1, attn2)

Sharding: 8 cores; core i handles batch b = i//4 and query-row block r = i%4
(1024 of 4096 rows). Each core computes the full normalized K for its batch
(recompute instead of collectives) and its row block of both sims + softmax.

ScalarE (ACT) is the bottleneck: the 8.4M softmax exps per core stream at
1 elem/cycle/lane @1.2GHz — a ~65us conveyor (measured 2.03us per 2048-chunk,
read-accumulator hidden under the next exp's pipe fill). Structure:
  - span = fixed ~6.5us prologue + phase-A ramp + exp conveyor + drain tail.
  - phase A must fully precede the conveyor (Abs_reciprocal_sqrt and Exp live
    in different ACT table sets; a mid-stream switch costs 2x2.7us, and ln+exp
    rsqrt in the exp set costs more than it saves — measured).
  - so phase A is ramp-optimized: chunks alternate between an ACT path
    (Square on ScalarE straight from PSUM) and a DVE path (cast+mul on
    VectorE) so no single engine gates the chunk cadence; the ars prime runs
    at t~0 and a dummy exp right after the last ars preloads the exp table
    during the ramp tail.
  - row sums via the ACT exp accumulator; attn normalize mul on VectorE;
    output fp16 (attn ~3e-4 sits mid fp16 normal range; 8x finer than bf16).
|sim| <= 1 because q/k are unit vectors, so softmax needs no max subtraction.
"""

import numpy as np
import ml_dtypes

B, C, H, W, D = 2, 256, 64, 64, 128
N = H * W  # 4096
QBLK = N // 4  # 1024 query rows per core
N_CORES = 8

_cached = {}


def _build():
    import concourse.mybir as mybir
    import concourse.tile as tile
    from concourse.tile_rust import add_dep_helper
    from concourse import bacc
    from contextlib import ExitStack

    f32 = mybir.dt.float32
    f16 = mybir.dt.float16
    bf16 = mybir.dt.bfloat16
    AF = mybir.ActivationFunctionType

    nc = bacc.Bacc(
        "TRN2",
        target_bir_lowering=False,
        debug=False,
        enable_asserts=False,
        num_devices=N_CORES,
    )

    f1_ext = nc.dram_tensor("f1", [C, N], bf16, kind="ExternalInput").ap()
    f2_ext = nc.dram_tensor("f2", [C, N], bf16, kind="ExternalInput").ap()
    xq_ext = nc.dram_tensor("xq", [C, QBLK], bf16, kind="ExternalInput").ap()
    wqkT_ext = nc.dram_tensor("wqkT", [C, D], bf16, kind="ExternalInput").ap()
    out_ext = nc.dram_tensor("out", [2, QBLK, N], f16, kind="ExternalOutput").ap()

    XCH = 1024  # phase A chunk (proj psum [128,1024] = 2 banks)
    CH = 2048  # phase B sim/exp chunk ([128,2048] = 4 banks)

    with tile.TileContext(nc) as tc, ExitStack() as ctx:
        consts = ctx.enter_context(tc.tile_pool(name="consts", bufs=1))
        xin = ctx.enter_context(tc.tile_pool(name="xin", bufs=8))
        ysq_pool = ctx.enter_context(tc.tile_pool(name="ysq", bufs=3))
        ybf_pool = ctx.enter_context(tc.tile_pool(name="ybf", bufs=3))
        rk_pool = ctx.enter_context(tc.tile_pool(name="rk", bufs=3))
        kn_pool = ctx.enter_context(tc.tile_pool(name="kn", bufs=1))
        e_pool = ctx.enter_context(tc.tile_pool(name="epool", bufs=8))
        attn_pool = ctx.enter_context(tc.tile_pool(name="attn", bufs=3))
        stat_pool = ctx.enter_context(tc.tile_pool(name="stat", bufs=4))

        # constants
        wqkT_sb = [
            consts.tile([128, D], bf16, tag=f"wqkT{k}", name=f"wqkT{k}")
            for k in range(2)
        ]
        nc.sync.dma_start(out=wqkT_sb[0][:], in_=wqkT_ext[0:128, :])
        nc.sync.dma_start(out=wqkT_sb[1][:], in_=wqkT_ext[128:256, :])
        ones_sb = consts.tile([128, 128], bf16, tag="ones", name="ones")
        nc.vector.memset(ones_sb[:], 1.0)
        # prime the ars table set at t~0 (abs_reciprocal_sqrt_and_small also
        # contains Square, so the whole phase A runs on this one set).
        warm = consts.tile([128, 1], f32, tag="warm", name="warm")
        nc.scalar.activation(out=warm[:], in_=ones_sb[:, 0:1], func=AF.Abs_reciprocal_sqrt)

        last_ars = None

        with tc.tile_pool(name="proj_psum", bufs=2, space="PSUM") as proj_psum, \
             tc.tile_pool(name="n2_psum", bufs=2, space="PSUM") as n2_psum:

            def norm_chunk(x_lo, x_hi, xn, h0, path):
                """project + column-l2-normalize one [128, XCH] chunk.

                path="act": Square on ScalarE straight from PSUM; normalize
                mul reads PSUM on VectorE. path="dve": cast+square+mul all on
                VectorE. Alternating paths keeps the chunk cadence from being
                bound by any one engine during the ramp.
                """
                nonlocal last_ars
                ps = proj_psum.tile([128, XCH], f32, tag="proj", name="pps")
                for c in range(XCH // 512):
                    sl = slice(c * 512, (c + 1) * 512)
                    nc.tensor.matmul(
                        ps[:, sl], wqkT_sb[0][:], x_lo[:, sl], start=True, stop=False
                    )
                    nc.tensor.matmul(
                        ps[:, sl], wqkT_sb[1][:], x_hi[:, sl], start=False, stop=True
                    )
                ysq = ysq_pool.tile([128, XCH], bf16, tag="ysq", name="ysq")
                if path == "act":
                    nc.scalar.activation(out=ysq[:], in_=ps[:], func=AF.Square)
                else:
                    y_bf = ybf_pool.tile([128, XCH], bf16, tag="ybf", name="y_bf")
                    nc.vector.tensor_copy(y_bf[:], ps[:])
                    nc.vector.tensor_mul(ysq[:], y_bf[:], y_bf[:])
                nps = n2_psum.tile([128, XCH], f32, tag="n2", name="nps")
                for c in range(XCH // 512):
                    sl = slice(c * 512, (c + 1) * 512)
                    nc.tensor.matmul(
                        nps[:, sl], ones_sb[:], ysq[:, sl], start=True, stop=True
                    )
                rk = rk_pool.tile([128, XCH], f16, tag="rk", name="rk")
                last_ars = nc.scalar.activation(
                    out=rk[:], in_=nps[:], func=AF.Abs_reciprocal_sqrt
                )
                if path == "act":
                    nc.vector.tensor_mul(xn[:, h0 : h0 + XCH], ps[:], rk[:])
                else:
                    nc.vector.tensor_mul(xn[:, h0 : h0 + XCH], y_bf[:], rk[:])

            def load_x(x_ext, h0):
                x_lo = xin.tile([128, XCH], bf16, tag="xin", name="x_lo")
                x_hi = xin.tile([128, XCH], bf16, tag="xin", name="x_hi")
                nc.sync.dma_start(out=x_lo[:], in_=x_ext[0:128, h0 : h0 + XCH])
                nc.sync.dma_start(out=x_hi[:], in_=x_ext[128:256, h0 : h0 + XCH])
                return x_lo, x_hi

            qn = kn_pool.tile([128, QBLK], bf16, tag="qn", name="qn")
            k1n = kn_pool.tile([128, N], bf16, tag="k1n", name="k1n")
            k2n = kn_pool.tile([128, N], bf16, tag="k2n", name="k2n")

            chunks = [(xq_ext, qn, 0)]
            chunks += [(f1_ext, k1n, h * XCH) for h in range(N // XCH)]
            chunks += [(f2_ext, k2n, h * XCH) for h in range(N // XCH)]
            for idx, (ext, xn, h0) in enumerate(chunks):
                x_lo, x_hi = load_x(ext, h0)
                norm_chunk(x_lo, x_hi, xn, h0, "act" if idx % 2 == 0 else "dve")

            # preload the exp table set during the ramp tail: dummy exp
            # ordered after the last ars so there are only 2 table loads.
            edum = consts.tile([128, 1], f16, tag="edum", name="edum")
            ex0 = nc.scalar.activation(out=edum[:], in_=warm[:], func=AF.Exp)
            add_dep_helper(
                ex0.ins, last_ars.ins, sync=False,
                reason="order all ars (ars table) before exp table load",
            )

        with tc.tile_pool(name="sim_psum", bufs=2, space="PSUM") as sim_psum:

            def phase_b(kn, s):
                """row block of sim + softmax for one K map, streamed to out[s]."""
                for t in range(QBLK // 128):
                    lhsT = qn[:, t * 128 : (t + 1) * 128]
                    attn = attn_pool.tile([128, N], f16, tag="attn", name="attn")
                    stile = stat_pool.tile([128, 2], f32, tag="stile", name="stile")
                    e_chunks = []
                    for j in range(N // CH):
                        ps = sim_psum.tile([128, CH], f32, tag="sim", name="sim_ps")
                        for c in range(CH // 512):
                            csl = slice(j * CH + c * 512, j * CH + (c + 1) * 512)
                            nc.tensor.matmul(
                                ps[:, c * 512 : (c + 1) * 512],
                                lhsT,
                                kn[:, csl],
                                start=True,
                                stop=True,
                            )
                        e = e_pool.tile([128, CH], f16, tag="e", name="e")
                        nc.scalar.activation(
                            out=e[:],
                            in_=ps[:],
                            func=AF.Exp,
                            accum_out=stile[:, j : j + 1],
                        )
                        e_chunks.append(e)
                    ssum = stat_pool.tile([128, 1], f32, tag="ssum", name="ssum")
                    nc.vector.reduce_sum(ssum[:], stile[:], axis=mybir.AxisListType.X)
                    recip = stat_pool.tile([128, 1], f32, tag="recip", name="recip")
                    nc.vector.reciprocal(recip[:], ssum[:])
                    last_tile = s == 1 and t == QBLK // 128 - 1
                    for j, e in enumerate(e_chunks):
                        # split the final chunk's normalize+store so the last
                        # DMA starts sooner (shorter drain tail)
                        nsp = 2 if (last_tile and j == len(e_chunks) - 1) else 1
                        w = CH // nsp
                        for u in range(nsp):
                            a0 = j * CH + u * w
                            nc.vector.tensor_scalar_mul(
                                attn[:, a0 : a0 + w], e[:, u * w : u * w + w], recip[:]
                            )
                            nc.sync.dma_start(
                                out=out_ext[
                                    s, t * 128 : (t + 1) * 128, a0 : a0 + w
                                ],
                                in_=attn[:, a0 : a0 + w],
                            )

            phase_b(k1n, 0)
            phase_b(k2n, 1)

    nc.compile()
    return nc


def _get_nc():
    if "nc" not in _cached:
        _cached["nc"] = _build()
    return _cached["nc"]


def _in_maps(fmap1, fmap2, dmap, Wqk):
    bf = ml_dtypes.bfloat16
    f1r = np.asarray(fmap1, dtype=np.float32).reshape(B, C, N)
    f2r = np.asarray(fmap2, dtype=np.float32).reshape(B, C, N)
    dqr = np.asarray(dmap, dtype=np.float32).reshape(B, C, N)
    wT = np.ascontiguousarray(np.asarray(Wqk, dtype=np.float32).T).astype(bf)

    in_maps = []
    for i in range(N_CORES):
        b, r = divmod(i, 4)
        in_maps.append(
            {
                "f1": np.ascontiguousarray(f1r[b]).astype(bf),
                "f2": np.ascontiguousarray(f2r[b]).astype(bf),
                "xq": np.ascontiguousarray(
                    dqr[b][:, r * QBLK : (r + 1) * QBLK]
                ).astype(bf),
                "wqkT": wT,
            }
        )
    return in_maps


def kernel(fmap1, fmap2, dmap, Wqk):
    from concourse.bass_utils import run_bass_kernel_spmd

    in_maps = _in_maps(fmap1, fmap2, dmap, Wqk)
    nc = _get_nc()
    res = run_bass_kernel_spmd(nc, in_maps, core_ids=list(range(N_CORES)))
    _cached["last_result"] = res

    attn1 = np.empty((B, 1, N, N), dtype=np.float32)
    attn2 = np.empty((B, 1, N, N), dtype=np.float32)
    for i in range(N_CORES):
        b, r = divmod(i, 4)
        o = res.results[i]["out"]
        attn1[b, 0, r * QBLK : (r + 1) * QBLK, :] = o[0].astype(np.float32)
        attn2[b, 0, r * QBLK : (r + 1) * QBLK, :] = o[1].astype(np.float32)
    return (attn1, attn2)
